# revision 19
# baseline (speedup 1.0000x reference)
"""Trainium2 Bass kernel for nn_CSGO_model (4-layer transformer + 26 MLP heads).

v2 over the staged baseline:
- LN rsqrt on DVE (quake seed + 2 Newton) -> no Ln/Exp act-table loads on the
  LN critical path; only 2 table loads per layer (gelu, exp), both warmed
  during GEMM phases.
- LN mean matmuls in float32r (1 cyc/row at 256 cols, vs 4 for fp32).
- LN stats matmuls interleaved into the producer loops (x-load, out-proj,
  ff2) so stats are ready right after the last producer chunk.
- Softmax denominator folded into the AV matmul (ones columns interleaved
  into vtok) -> the 2 dn matmuls per head are gone.
- The x_comb AllGather collective (43us) replaced by peer remote_dma
  broadcasts (XOR-slot addressing, core-id stamped in-band so the host
  unshard is robust to the ucode lane->slot permutation).
- Head w3 (1-col matmuls, 1.1us each) replaced by DVE multiply + one
  256-col ones-matmul per slot.
- Heads read the exchange buffer directly as matmul rhs (no xcombT copies).
- Deeper weight-stream prefetch pools.
"""
import sys
import types

sys.path.insert(0, '/opt/trn_rl_repo')

if 'antenv.axon_hooks' not in sys.modules:
    try:
        from antenv import axon_hooks  # noqa: F401
    except ImportError:
        _hookmod = types.ModuleType('antenv.axon_hooks')
        _hookmod.set_axon_ntff_profile_hook = lambda h: None
        _hookmod.get_axon_ntff_profile_hook = lambda: None
        sys.modules['antenv.axon_hooks'] = _hookmod

import numpy as np
import ml_dtypes

BF16 = ml_dtypes.bfloat16

# Model dims
D = 1024
NHEADS = 16
HD = 64
INNER = NHEADS * HD
FF = 2048
L = 4
NOUT = 26
IDH = 512
B = 64
T = 32

N_CORES = 8
B_LOC = B // N_CORES          # 8 batches per core
M = B_LOC * T                 # 256 tokens per core
M4 = 2 * B_LOC                # 16 token columns used in layer 4 (t in {0,1})
DCH = D // 128                # 8 feature chunks
FCH = FF // 128               # 16
H_SLOTS = 4                   # padded head slots per core (8*4=32 >= 26)

N_WARM = 16                   # keep-warm dummy matmuls per LN site

_CACHE = {}


# ---------------------------------------------------------------- device code

def _build_nc():
    import concourse.tile as tile
    from concourse import mybir, bacc

    f32 = mybir.dt.float32
    f32r = mybir.dt.float32r
    bf16 = mybir.dt.bfloat16
    u32 = mybir.dt.uint32
    Alu = mybir.AluOpType
    Act = mybir.ActivationFunctionType
    import os
    GELU = Act.Tanh if os.environ.get('K_SIM_TANH') else Act.Gelu

    nc = bacc.Bacc("TRN2", target_bir_lowering=False, debug=False,
                   num_devices=N_CORES)

    # ------------- DRAM tensors (per-core inputs, host-prepared layouts)
    x_d = nc.dram_tensor("x", [DCH, 128, M], f32, kind="ExternalInput")
    wqk_d = nc.dram_tensor("wqk", [L, 16, 128, DCH, 128], bf16,
                           kind="ExternalInput")
    wv_d = nc.dram_tensor("wv", [L, DCH, 128, INNER], bf16,
                          kind="ExternalInput")
    wo_d = nc.dram_tensor("wo", [L, DCH, 128, DCH, 128], bf16,
                          kind="ExternalInput")
    wf1_d = nc.dram_tensor("wf1", [L, FCH, 128, DCH, 128], bf16,
                           kind="ExternalInput")
    wf2_d = nc.dram_tensor("wf2", [L, DCH, 128, FCH, 128], bf16,
                           kind="ExternalInput")
    biasp_d = nc.dram_tensor("biasp", [128, L, 32], f32, kind="ExternalInput")
    qkbp_d = nc.dram_tensor("qkbp", [128, L, 2 * DCH], f32,
                            kind="ExternalInput")
    jones_d = nc.dram_tensor("jones", [128, 128], bf16, kind="ExternalInput")
    jones1_d = nc.dram_tensor("jones1", [128, 128], bf16,
                              kind="ExternalInput")
    mask01_d = nc.dram_tensor("mask01", [128, 2, M], bf16,
                              kind="ExternalInput")
    mask01t_d = nc.dram_tensor("mask01t", [128, 2, M4], bf16,
                               kind="ExternalInput")
    cid_d = nc.dram_tensor("cid", [1, 1], bf16, kind="ExternalInput")
    hw1_d = nc.dram_tensor("hw1", [H_SLOTS, 2 * DCH, 128, IDH], bf16,
                           kind="ExternalInput")
    hw2_d = nc.dram_tensor("hw2", [H_SLOTS, 4, 128, IDH], bf16,
                           kind="ExternalInput")
    hw3p_d = nc.dram_tensor("hw3p", [128, H_SLOTS * 4], bf16,
                            kind="ExternalInput")
    hb1p_d = nc.dram_tensor("hb1p", [128, H_SLOTS, 4], f32,
                            kind="ExternalInput")
    hb2p_d = nc.dram_tensor("hb2p", [128, H_SLOTS, 4], f32,
                            kind="ExternalInput")
    hb3p_d = nc.dram_tensor("hb3p", [1, H_SLOTS], f32, kind="ExternalInput")

    out_d = nc.dram_tensor("out_h", [1, H_SLOTS, B], f32,
                           kind="ExternalOutput")
    ids_d = nc.dram_tensor("ids_out", [1, N_CORES], f32,
                           kind="ExternalOutput")

    XW = 136                      # exchange payload width: 128 data + id + pad

    with tile.TileContext(nc) as tc:
        from contextlib import ExitStack
        with ExitStack() as ctx:
            const = ctx.enter_context(tc.tile_pool(name="const", bufs=1))
            ps_a = ctx.enter_context(
                tc.tile_pool(name="ps_a", bufs=7, space="PSUM"))
            ps_ln = ctx.enter_context(
                tc.tile_pool(name="ps_ln", bufs=1, space="PSUM"))
            # head pools outlive the transformer pools -> created first
            w1h_pool = ctx.enter_context(tc.tile_pool(name="w1h", bufs=3))
            w2h_pool = ctx.enter_context(tc.tile_pool(name="w2h", bufs=2))
            hact = ctx.enter_context(tc.tile_pool(name="hact", bufs=2))
            tfs = ctx.enter_context(ExitStack())
            hres = tfs.enter_context(tc.tile_pool(name="hres", bufs=1))
            sq = tfs.enter_context(tc.tile_pool(name="sq", bufs=4))
            stats = tfs.enter_context(tc.tile_pool(name="stats", bufs=8))
            actb = tfs.enter_context(tc.tile_pool(name="actb", bufs=2))
            qkp = tfs.enter_context(tc.tile_pool(name="qkp", bufs=1))
            attp = tfs.enter_context(tc.tile_pool(name="attp", bufs=6))
            obufp = tfs.enter_context(tc.tile_pool(name="obufp", bufs=1))
            g1p = tfs.enter_context(tc.tile_pool(name="g1p", bufs=1))
            wqk_pool = tfs.enter_context(tc.tile_pool(name="wqk", bufs=6))
            wv_pool = tfs.enter_context(tc.tile_pool(name="wv", bufs=8, ))
            wo_pool = tfs.enter_context(tc.tile_pool(name="wo", bufs=3))
            wf1_pool = tfs.enter_context(tc.tile_pool(name="wf1", bufs=6))
            wf2_pool = tfs.enter_context(tc.tile_pool(name="wf2", bufs=3))

            # constants
            jones = const.tile([128, 128], bf16, tag="jones")
            nc.sync.dma_start(jones[:], jones_d[:])
            jones1 = const.tile([128, 128], bf16, tag="jones1")
            nc.sync.dma_start(jones1[:], jones1_d[:])
            mask01 = const.tile([128, 2, M], bf16, tag="mask01")
            nc.sync.dma_start(mask01[:], mask01_d[:])
            mask01t = const.tile([128, 2, M4], bf16, tag="mask01t")
            nc.sync.dma_start(mask01t[:], mask01t_d[:])
            qkbp = const.tile([128, L, 2 * DCH], f32, tag="qkbp")
            nc.sync.dma_start(qkbp[:], qkbp_d[:])
            biasp = const.tile([128, L, 32], f32, tag="biasp")
            nc.sync.dma_start(biasp[:], biasp_d[:])
            eps0 = const.tile([128, 1], f32, tag="eps0")
            nc.vector.memset(eps0[:], 1e-6)
            eps1 = const.tile([128, 1], f32, tag="eps1")
            nc.vector.memset(eps1[:], 1e-5)
            # fp32 per-partition scalar for the quake rsqrt seed
            c_magic = const.tile([128, 1], f32, tag="c_magic")
            nc.vector.memset(c_magic[:], float(0x5f3759df))
            # tiny dummy activations force act-table loads off-path
            awrm = const.tile([128, 1], f32, tag="awrm")

            def act_table_warm(*funcs):
                for fn in funcs:
                    nc.scalar.activation(awrm[:], eps1[:], fn)

            act_table_warm(Act.Exp)       # set 0: exp/square/identity/copy

            # exchange buffers
            x16e = const.tile([128, XW], bf16, tag="x16e")
            x16v = x16e[:, 0:128].rearrange("p (c tt b) -> p c tt b",
                                            c=DCH, tt=2)
            nc.sync.dma_start(x16e[0:1, 128:129], cid_d[:])
            gsbx = const.tile([128, N_CORES, XW], bf16, tag="gsbx")

            # vtok with interleaved ones columns: [:, i, hh, 0:64] = V,
            # [:, i, hh, 64:128] = 1.0 -> AV matmul rows 64:128 produce the
            # softmax denominator for free.
            vtok1 = const.tile([128, 2, NHEADS, 128], bf16, tag="vtok1")
            nc.vector.memset(vtok1[:, :, :, 64:128], 1.0)

            # residual, feature-major [128p, chunk, token], fp32
            h = hres.tile([128, DCH, M], f32, tag="h")

            w1h_tiles, w2h_tiles = [], []

            def warm(n):
                if n <= 0:
                    return
                ps_w = ps_a.tile([128, 2, M], f32, tag="a", name=None)
                for wi in range(n):
                    nc.tensor.matmul(ps_w[:, wi % 2, :], jones[:],
                                     mask01[:, 0, :],
                                     start=True, stop=True)

            def stats_psum(mcols):
                return ps_ln.tile([128, 2, mcols], f32, tag="ln",
                                  name=f"ln_st_{nc.next_id()}")

            def emit_stats(ps_st, c, src_c, mcols, mtag):
                """Accumulate [sum(h); sum(h^2)] for one produced chunk c
                into ps_st via a single 512-col bf16 matmul (ONE psum group
                per bank -- interleaved groups in one 2KB zero region are
                unsafe). ACT makes the bf16 copy + square off-path."""
                hbs = sq.tile([128, 2, mcols], bf16, tag="sqb" + mtag,
                              name=f"hbs_{nc.next_id()}")
                nc.scalar.activation(hbs[:, 0, :], src_c, Act.Copy)
                nc.scalar.activation(hbs[:, 1, :], src_c, Act.Square)
                nc.tensor.matmul(ps_st[:], jones[:], hbs[:],
                                 start=(c == 0), stop=(c == DCH - 1))

            def ln_tail(ps_st, site, src, mcols, mtag):
                """DVE-only rsqrt + xln writes. src [128, DCH, mcols] fp32.
                Returns xln [128, DCH, mcols] bf16."""
                eps = eps0 if site == 0 else eps1
                warm(N_WARM)
                st = lambda: stats.tile([128, mcols], f32, tag="st" + mtag,
                                        name=f"st_{nc.next_id()}")
                # mu_s is read until the end of the xln writes -- it must NOT
                # share the chain temps' rotation window.
                mu_s = stats.tile([128, mcols], f32, tag="mu" + mtag, bufs=2,
                                  name=f"mu_{nc.next_id()}")
                nc.vector.tensor_copy(mu_s[:], ps_st[:, 0, :])
                mu = mu_s[:]
                mu2 = st()
                nc.vector.tensor_tensor(mu2[:], mu_s[:], ps_st[:, 0, :],
                                        Alu.mult)
                w = st()
                nc.vector.scalar_tensor_tensor(
                    w[:], ps_st[:, 1, :], eps[:], mu2[:], Alu.add,
                    Alu.subtract)
                # quake seed: y0_bits = 0x5f3759df - (w_bits >> 1), done in
                # the fp32 value domain (DVE converts u32 operands by value):
                # bits*-0.5 == -(bits>>1) up to <=1 lsb -- harmless for a
                # Newton seed.
                y = st()
                tneg = st()
                nc.vector.tensor_scalar(
                    tneg[:], w[:].bitcast(u32), -0.5, None, Alu.mult)
                nc.vector.tensor_scalar(
                    y[:].bitcast(u32), tneg[:], c_magic[:], None, Alu.add)
                # 2 Newton iterations: y <- y * (1.5 - 0.5 * w * y^2)
                for _ in range(2):
                    y2 = st()
                    nc.vector.tensor_mul(y2[:], y[:], y[:])
                    cc = st()
                    nc.vector.tensor_mul(cc[:], y2[:], w[:])
                    nc.vector.tensor_scalar(cc[:], cc[:], -0.5, 1.5,
                                            Alu.mult, Alu.add)
                    yn = st()
                    nc.vector.tensor_mul(yn[:], cc[:], y[:])
                    y = yn
                alpha = y
                xln = actb.tile([128, DCH, mcols], bf16, tag="xln" + mtag)
                for c0 in range(0, DCH, 2):
                    c1 = c0 + 2
                    tt_full = sq.tile([128, 2, mcols], f32, tag="sqf" + mtag)
                    mu_b = mu.unsqueeze(1).to_broadcast([128, 2, mcols])
                    al_b = alpha[:].unsqueeze(1).to_broadcast([128, 2, mcols])
                    nc.vector.tensor_sub(tt_full[:], src[:, c0:c1, :], mu_b)
                    nc.vector.tensor_mul(xln[:, c0:c1, :], tt_full[:], al_b)
                return xln

            # ---- load x (producer of layer-1 h) with interleaved LN1 stats
            ps_st = stats_psum(M)
            for c in range(DCH):
                nc.sync.dma_start(h[:, c, :], x_d[c])
                emit_stats(ps_st, c, h[:, c, :], M, "")

            for l in range(L):
                last = (l == L - 1)
                mq = M4 if last else M
                msk = mask01t if last else mask01

                # ---- attn pre-LN tail (stats already accumulated)
                xln = ln_tail(ps_st, 0, h[:], M, "")

                if last:
                    xln_q = actb.tile([128, DCH, M4], bf16, tag="xlnq")
                    nc.gpsimd.tensor_copy(
                        xln_q[:],
                        xln[:].rearrange("p c (b t) -> p c b t",
                                         b=B_LOC)[:, :, :, 0:2])

                # ---- Q,K feature-major GEMM -> qk [128, 2*DCH, M]
                qk = qkp.tile([128, 2 * DCH, M], bf16, tag="qk")
                for np_ in range(DCH):
                    ps = ps_a.tile([128, 2, M], f32, tag="a")
                    for i in range(2):
                        n = 2 * np_ + i
                        wt = wqk_pool.tile([128, DCH, 128], bf16, tag="wqk")
                        nc.sync.dma_start(wt[:], wqk_d[l, n])
                        if last and n < DCH:
                            for c in range(DCH):
                                nc.tensor.matmul(ps[:, i, 0:M4],
                                                 wt[:, c, :], xln_q[:, c, :],
                                                 start=(c == 0),
                                                 stop=(c == DCH - 1))
                        else:
                            for c in range(DCH):
                                nc.tensor.matmul(ps[:, i, :],
                                                 wt[:, c, :], xln[:, c, :],
                                                 start=(c == 0),
                                                 stop=(c == DCH - 1))
                    for i in range(2):
                        n = 2 * np_ + i
                        w_cols = M4 if (last and n < DCH) else M
                        nc.scalar.activation(qk[:, n, 0:w_cols],
                                             ps[:, i, 0:w_cols],
                                             Act.Identity,
                                             bias=qkbp[:, l, n:n + 1])

                # ---- V token-major GEMM -> vtok1[:, mc, hh, 0:64]
                wv_keep = []
                for c in range(DCH):
                    wvt = wv_pool.tile([128, INNER], bf16, tag="wv")
                    nc.sync.dma_start(wvt[:], wv_d[l, c])
                    wv_keep.append(wvt)
                for mc in range(2):
                    for jb in range(2):
                        psv = ps_a.tile([128, 512], f32, tag="a")
                        for c in range(DCH):
                            nc.tensor.matmul(
                                psv[:],
                                xln[:, c, mc * 128:(mc + 1) * 128],
                                wv_keep[c][:, jb * 512:(jb + 1) * 512],
                                start=(c == 0), stop=(c == DCH - 1))
                        nc.scalar.activation(
                            vtok1[:, mc, 8 * jb:8 * jb + 8, 0:64],
                            psv[:].rearrange("p (hh d) -> p hh d", hh=8),
                            Act.Copy)

                # ---- attention, head-pair by head-pair
                obuf = obufp.tile([128, DCH, M], bf16, tag="obuf")

                def emit_s(hc):
                    es = []
                    for j in range(2):
                        e_j = attp.tile([128, 2, mq], bf16, tag="e",
                                        name=f"e_{l}_{hc}_{j}")
                        es.append(e_j)
                    pss = []
                    for j in range(2):
                        ps_j = ps_a.tile([128, 2, mq], f32, tag="a",
                                         name=f"pss_{l}_{hc}_{j}")
                        pss.append(ps_j)
                    for i in range(2):
                        for j in range(2):
                            hp = j * 64
                            nc.tensor.matmul(
                                pss[j][:, i, :],
                                qk[hp:hp + 64, DCH + hc,
                                   i * 128:(i + 1) * 128],
                                qk[hp:hp + 64, hc, 0:mq],
                                start=True, stop=True)
                    for j in range(2):
                        nc.scalar.activation(es[j][:], pss[j][:], Act.Exp,
                                             scale=0.125)
                    for j in range(2):
                        nc.vector.tensor_mul(es[j][:], es[j][:], msk[:])
                    return es

                es_pend = [emit_s(0)]
                for hc in range(DCH):
                    if hc + 1 < DCH:
                        es_pend.append(emit_s(hc + 1))
                    es = es_pend.pop(0)
                    for j in range(2):
                        hh = 2 * hc + j
                        hp = j * 64
                        e = es[j]
                        ps_do = ps_a.tile([128, 2, mq], f32, tag="a")
                        for i in range(2):
                            nc.tensor.matmul(ps_do[:, 0, :], jones1[:],
                                             e[:, i, :],
                                             start=(i == 0), stop=(i == 1))
                        rd = stats.tile([128, mq], f32, tag="strd", bufs=4)
                        nc.vector.reciprocal_approx_fast(rd[0:64, :],
                                                         ps_do[0:64, 0, :])
                        for i in range(2):
                            nc.tensor.matmul(
                                ps_do[hp:hp + 64, 1, :],
                                vtok1[:, i, hh, 0:64],
                                e[:, i, :],
                                start=(i == 0), stop=(i == 1),
                                tile_position=(0, hp))
                        nc.vector.tensor_tensor(
                            obuf[hp:hp + 64, hc, 0:mq],
                            ps_do[hp:hp + 64, 1, :],
                            rd[0:64, :], Alu.mult)


                # ---- output projection (+ residual + out_b)
                # LN2 stats interleave with the drain of each chunk
                if last:
                    h_mid = hres.tile([128, DCH, M4], f32, tag="hmid")
                    h_t01 = h[:].rearrange("p c (b t) -> p c b t",
                                           b=B_LOC)[:, :, :, 0:2]
                ps_st = stats_psum(mq)
                for n in range(DCH):
                    wot = wo_pool.tile([128, DCH, 128], bf16, tag="wo")
                    nc.sync.dma_start(wot[:], wo_d[l, n])
                    ps = ps_a.tile([128, mq], f32, tag="a")
                    for c in range(DCH):
                        nc.tensor.matmul(ps[:],
                                         wot[:, c, :],
                                         obuf[:, c, 0:mq],
                                         start=(c == 0), stop=(c == DCH - 1))
                    if last:
                        nc.vector.scalar_tensor_tensor(
                            h_mid[:, n, :], ps[:], biasp[:, l, n:n + 1],
                            h_t01[:, n], Alu.add, Alu.add)
                        emit_stats(ps_st, n, h_mid[:, n, :], mq, "t")
                    else:
                        nc.vector.scalar_tensor_tensor(
                            h[:, n, :], ps[:], biasp[:, l, n:n + 1],
                            h[:, n, :], Alu.add, Alu.add)
                        emit_stats(ps_st, n, h[:, n, :], mq, "")

                # ---- ff pre-LN tail
                if last:
                    xln2 = ln_tail(ps_st, 1, h_mid[:], M4, "t")
                else:
                    xln2 = ln_tail(ps_st, 1, h[:], M, "")

                # ---- ff1 + gelu(x + b1) -> g1 [128, FCH, mq]
                act_table_warm(GELU)
                g1 = g1p.tile([128, FCH, M], bf16, tag="g1")
                for n in range(FCH):
                    wft = wf1_pool.tile([128, DCH, 128], bf16, tag="wf1")
                    nc.sync.dma_start(wft[:], wf1_d[l, n])
                    ps = ps_a.tile([128, mq], f32, tag="a")
                    for c in range(DCH):
                        nc.tensor.matmul(ps[:],
                                         wft[:, c, :],
                                         xln2[:, c, :],
                                         start=(c == 0), stop=(c == DCH - 1))
                    nc.scalar.activation(g1[:, n, 0:mq], ps[:], GELU,
                                         bias=biasp[:, l, 8 + n:9 + n])
                if not last:
                    act_table_warm(Act.Exp)   # back to set 0 for next layer

                # ---- ff2 (+ residual + b2); layer 4 writes x16e instead
                if last:
                    hm_v = h_mid[:].rearrange("p c (b t) -> p c b t", b=B_LOC)
                else:
                    ps_st = stats_psum(M)   # next layer's LN1 stats
                for n in range(DCH):
                    wft = wf2_pool.tile([128, FCH, 128], bf16, tag="wf2")
                    nc.sync.dma_start(wft[:], wf2_d[l, n])
                    ps = ps_a.tile([128, mq], f32, tag="a")
                    for c in range(FCH):
                        nc.tensor.matmul(ps[:],
                                         wft[:, c, :],
                                         g1[:, c, 0:mq],
                                         start=(c == 0), stop=(c == FCH - 1))
                    if last:
                        nc.vector.scalar_tensor_tensor(
                            x16v[:, n, :, :].rearrange("p tt b -> p b tt"),
                            ps[:], biasp[:, l, 24 + n:25 + n],
                            hm_v[:, n], Alu.add, Alu.add)
                    else:
                        nc.vector.scalar_tensor_tensor(
                            h[:, n, :], ps[:], biasp[:, l, 24 + n:25 + n],
                            h[:, n, :], Alu.add, Alu.add)
                        emit_stats(ps_st, n, h[:, n, :], M, "")

                # prefetch head weights during layers 2 and 3
                if l in (1, 2):
                    for sn in (2 * (l - 1), 2 * (l - 1) + 1):
                        w1h = w1h_pool.tile([128, 2 * DCH, IDH], bf16,
                                            tag="w1h")
                        nc.sync.dma_start(
                            w1h[:], hw1_d[sn].rearrange("c p n2 -> p c n2"))
                        w2h = w2h_pool.tile([128, 4, IDH], bf16, tag="w2h")
                        nc.sync.dma_start(
                            w2h[:], hw2_d[sn].rearrange("c p n2 -> p c n2"))
                        w1h_tiles.append(w1h)
                        w2h_tiles.append(w2h)

            # ---------------- peer-DMA exchange of x16e ----------------
            tfs.close()   # free transformer pools for the heads stage
            rsem = nc.alloc_semaphore("xchg_recv")
            lsem = nc.alloc_semaphore("xchg_sent")
            psem = nc.alloc_semaphore("xchg_prep")
            with tc.tile_critical():
                nc.gpsimd.bir_kernel_barrier_wait([list(range(N_CORES))])
                for d in range(N_CORES):
                    dests = [None] * 8
                    dests[d] = (0, d)
                    nc.gpsimd.remote_dma_broadcast(
                        gsbx[:, d, :], x16e[:], rsem, lsem,
                        rdests=dests).then_inc(psem, 1)
                nc.gpsimd.wait_ge(psem, N_CORES)
                nc.gpsimd.trigger_dma(count=N_CORES)
                nc.gpsimd.wait_ge(rsem, 2 * N_CORES)

            # core-id stamps -> host unshard mapping
            ids_sb = const.tile([1, N_CORES], f32, tag="ids_sb")
            nc.vector.tensor_copy(ids_sb[:],
                                  gsbx[0:1, :, 128:129]
                                  .rearrange("p j o -> p (j o)"))
            nc.sync.dma_start(ids_d[:], ids_sb[:])

            # PE warm-up after the exchange idle gap
            ps_w = ps_a.tile([128, 2, M], f32, tag="a", name="warmup_ps")
            for wi in range(12):
                nc.tensor.matmul(ps_w[:, wi % 2, :], jones[:],
                                 mask01[:, 0, :],
                                 start=True, stop=True)

            # ---------------- 26 (padded 32) MLP heads, feature-major ------
            hb1p = const.tile([128, H_SLOTS, 4], f32, tag="hb1p")
            nc.sync.dma_start(hb1p[:], hb1p_d[:])
            hb2p = const.tile([128, H_SLOTS, 4], f32, tag="hb2p")
            nc.sync.dma_start(hb2p[:], hb2p_d[:])
            hb3p = const.tile([1, H_SLOTS], f32, tag="hb3p")
            nc.sync.dma_start(hb3p[:], hb3p_d[:])
            hw3 = const.tile([128, H_SLOTS * 4], bf16, tag="hw3")
            nc.sync.dma_start(hw3[:], hw3p_d[:])
            outacc = const.tile([1, H_SLOTS, B], f32, tag="outacc")

            def xcomb_rhs(kc):
                # [128, slot j, batch i] direct from the exchange buffer;
                # slot-local batch order, host remaps via ids_out.
                tt, c = kc // DCH, kc % DCH
                off = c * 16 + tt * 8
                return gsbx[:, :, off:off + 8]

            for n in range(H_SLOTS):
                w1h = w1h_tiles[n]
                w2h = w2h_tiles[n]
                ps1 = ps_a.tile([128, 4, B], f32, tag="a")
                for nc2 in range(4):
                    for kc in range(2 * DCH):
                        nc.tensor.matmul(
                            ps1[:, nc2, :],
                            w1h[:, kc, nc2 * 128:(nc2 + 1) * 128],
                            xcomb_rhs(kc),
                            start=(kc == 0), stop=(kc == 2 * DCH - 1))
                h1t = hact.tile([128, 4, B], bf16, tag="h1t")
                for nc2 in range(4):
                    nc.scalar.activation(h1t[:, nc2, :], ps1[:, nc2, :],
                                         Act.Relu,
                                         bias=hb1p[:, n, nc2:nc2 + 1])
                ps2 = ps_a.tile([128, 4, B], f32, tag="a")
                for nc2 in range(4):
                    for kc in range(4):
                        nc.tensor.matmul(
                            ps2[:, nc2, :],
                            w2h[:, kc, nc2 * 128:(nc2 + 1) * 128],
                            h1t[:, kc, :],
                            start=(kc == 0), stop=(kc == 3))
                h2t = hact.tile([128, 4, B], bf16, tag="h2t")
                for nc2 in range(4):
                    nc.scalar.activation(h2t[:, nc2, :], ps2[:, nc2, :],
                                         Act.Relu,
                                         bias=hb2p[:, n, nc2:nc2 + 1])
                # w3: DVE multiply + one 256-col ones-matmul
                h2w = hact.tile([128, 4, B], bf16, tag="h2w")
                nc.vector.tensor_mul(
                    h2w[:], h2t[:],
                    hw3[:, n * 4:(n + 1) * 4].unsqueeze(2)
                    .to_broadcast([128, 4, B]))
                ps3 = ps_a.tile([1, 4, B], f32, tag="a")
                nc.tensor.matmul(ps3[:].rearrange("p k b -> p (k b)"),
                                 jones1[:, 0:1],
                                 h2w[:].rearrange("p k b -> p (k b)"),
                                 start=True, stop=True)
                s3 = hact.tile([1, 4, B], f32, tag="s3")
                nc.vector.tensor_copy(s3[:], ps3[:])
                o01 = hact.tile([1, 2, B], f32, tag="o01")
                nc.vector.tensor_tensor(o01[:, 0, :], s3[:, 0, :],
                                        s3[:, 1, :], Alu.add)
                nc.vector.tensor_tensor(o01[:, 1, :], s3[:, 2, :],
                                        s3[:, 3, :], Alu.add)
                nc.vector.scalar_tensor_tensor(
                    outacc[:, n, :], o01[:, 0, :], hb3p[0:1, n:n + 1],
                    o01[:, 1, :], Alu.add, Alu.add)

            nc.sync.dma_start(out_d[:], outacc[:])

    nc.finalize()
    return nc


# ---------------------------------------------------------------- host side

def _prep_in_maps(inputs):
    x = np.asarray(inputs['x'], np.float32)
    qkv_w = np.asarray(inputs['qkv_w'], np.float32)
    out_w = np.asarray(inputs['out_w'], np.float32)
    out_b = np.asarray(inputs['out_b'], np.float32)
    attn_ln_g = np.asarray(inputs['attn_ln_g'], np.float32)
    attn_ln_b = np.asarray(inputs['attn_ln_b'], np.float32)
    ff_ln_g = np.asarray(inputs['ff_ln_g'], np.float32)
    ff_ln_b = np.asarray(inputs['ff_ln_b'], np.float32)
    ff_w1 = np.asarray(inputs['ff_w1'], np.float32)
    ff_b1 = np.asarray(inputs['ff_b1'], np.float32)
    ff_w2 = np.asarray(inputs['ff_w2'], np.float32)
    ff_b2 = np.asarray(inputs['ff_b2'], np.float32)
    head_w1 = np.asarray(inputs['head_w1'], np.float32)
    head_b1 = np.asarray(inputs['head_b1'], np.float32)
    head_w2 = np.asarray(inputs['head_w2'], np.float32)
    head_b2 = np.asarray(inputs['head_b2'], np.float32)
    head_w3 = np.asarray(inputs['head_w3'], np.float32)
    head_b3 = np.asarray(inputs['head_b3'], np.float32)

    # Fold the LN affine transform into the following GEMM weights (exact)
    ag_eff = attn_ln_g * np.float32((1.0 + 1e-5) ** -0.5)
    qkvb = np.einsum('ld,ldn->ln', attn_ln_b, qkv_w)
    ff_b1 = ff_b1 + np.einsum('ld,ldn->ln', ff_ln_b, ff_w1)
    qkv_w = qkv_w * ag_eff[:, :, None]
    ff_w1 = ff_w1 * ff_ln_g[:, :, None]
    vbias = qkvb[:, 2 * INNER:]
    out_b = out_b + np.einsum('lk,lkd->ld', vbias, out_w)

    wqk = np.zeros((L, 16, 128, DCH, 128), np.float32)
    wv = np.zeros((L, DCH, 128, INNER), np.float32)
    wo = np.zeros((L, DCH, 128, DCH, 128), np.float32)
    wf1 = np.zeros((L, FCH, 128, DCH, 128), np.float32)
    wf2 = np.zeros((L, DCH, 128, FCH, 128), np.float32)
    for l in range(L):
        wqk[l] = qkv_w[l][:, :2 * INNER].reshape(
            DCH, 128, 16, 128).transpose(2, 1, 0, 3)
        wv[l] = qkv_w[l][:, 2 * INNER:].reshape(DCH, 128, INNER)
        wo[l] = out_w[l].reshape(DCH, 128, DCH, 128).transpose(2, 1, 0, 3)
        wf1[l] = ff_w1[l].reshape(DCH, 128, FCH, 128).transpose(2, 1, 0, 3)
        wf2[l] = ff_w2[l].reshape(FCH, 128, DCH, 128).transpose(2, 1, 0, 3)
    wqk = np.ascontiguousarray(wqk).astype(BF16)
    wv = np.ascontiguousarray(wv).astype(BF16)
    wo = np.ascontiguousarray(wo).astype(BF16)
    wf1 = np.ascontiguousarray(wf1).astype(BF16)
    wf2 = np.ascontiguousarray(wf2).astype(BF16)

    biasp = np.zeros((128, L, 32), np.float32)
    biasp[:, :, 0:8] = out_b.reshape(L, 8, 128).transpose(2, 0, 1)
    biasp[:, :, 8:24] = ff_b1.reshape(L, 16, 128).transpose(2, 0, 1)
    biasp[:, :, 24:32] = ff_b2.reshape(L, 8, 128).transpose(2, 0, 1)

    qkbp = np.ascontiguousarray(
        qkvb[:, :2 * INNER].reshape(L, 2 * DCH, 128).transpose(2, 0, 1))

    jones = np.full((128, 128), 1.0 / D, np.float32).astype(BF16)
    jones1 = np.ones((128, 128), np.float32).astype(BF16)

    mask01 = np.zeros((128, 2, M), np.float32)
    for i in range(2):
        for p in range(128):
            kb = (i * 128 + p) // T
            mask01[p, i, kb * T:(kb + 1) * T] = 1.0
    mask01t = np.zeros((128, 2, M4), np.float32)
    for i in range(2):
        for p in range(128):
            kb = (i * 128 + p) // T
            mask01t[p, i, kb * 2:(kb + 1) * 2] = 1.0
    mask01 = mask01.astype(BF16)
    mask01t = mask01t.astype(BF16)

    in_maps = []
    for c in range(N_CORES):
        xs = x[c * B_LOC:(c + 1) * B_LOC].reshape(M, D)
        x_fm = np.ascontiguousarray(xs.T.reshape(DCH, 128, M))

        hw1 = np.zeros((H_SLOTS, 2 * DCH, 128, IDH), np.float32)
        hw2 = np.zeros((H_SLOTS, 4, 128, IDH), np.float32)
        hw3p = np.zeros((128, H_SLOTS * 4), np.float32)
        hb1p = np.zeros((128, H_SLOTS, 4), np.float32)
        hb2p = np.zeros((128, H_SLOTS, 4), np.float32)
        hb3p = np.zeros((1, H_SLOTS), np.float32)
        for n in range(H_SLOTS):
            g = n * N_CORES + c
            if g >= NOUT:
                continue
            hw1[n] = head_w1[g].reshape(2 * DCH, 128, IDH)
            hw2[n] = head_w2[g].reshape(4, 128, IDH)
            hw3p[:, n * 4:(n + 1) * 4] = head_w3[g].reshape(4, 128).T
            hb1p[:, n, :] = head_b1[g].reshape(4, 128).T
            hb2p[:, n, :] = head_b2[g].reshape(4, 128).T
            hb3p[0, n] = head_b3[g, 0]
        in_maps.append({
            'x': x_fm,
            'wqk': wqk, 'wv': wv, 'wo': wo, 'wf1': wf1, 'wf2': wf2,
            'biasp': biasp, 'qkbp': qkbp,
            'jones': jones, 'jones1': jones1,
            'mask01': mask01, 'mask01t': mask01t,
            'cid': np.full((1, 1), float(c), np.float32).astype(BF16),
            'hw1': hw1.astype(BF16), 'hw2': hw2.astype(BF16),
            'hw3p': hw3p.astype(BF16),
            'hb1p': hb1p, 'hb2p': hb2p, 'hb3p': hb3p,
        })
    return in_maps


def _get_nc():
    if 'nc' not in _CACHE:
        _CACHE['nc'] = _build_nc()
    return _CACHE['nc']


def _unshard_out(results):
    out = np.zeros((B, NOUT, 1), np.float32)
    for c in range(N_CORES):
        oh = results[c]['out_h']           # [1, H_SLOTS, B] slot-local order
        ids = np.rint(np.asarray(results[c]['ids_out'],
                                 np.float32)).astype(int)[0]  # [N_CORES]
        for n in range(H_SLOTS):
            g = n * N_CORES + c
            if g >= NOUT:
                continue
            for j in range(N_CORES):
                s = ids[j]                  # source core whose batches sit
                out[s * B_LOC:(s + 1) * B_LOC, g, 0] = \
                    oh[0, n, j * B_LOC:(j + 1) * B_LOC]
    return out


def kernel(**inputs):
    from concourse.bass_utils import run_bass_kernel_spmd
    nc = _get_nc()
    in_maps = _prep_in_maps(inputs)
    res = run_bass_kernel_spmd(nc, in_maps, core_ids=list(range(N_CORES)))
    return _unshard_out(res.results)
